# revision 7
# baseline (speedup 1.0000x reference)
"""DeepseekMoE on 8 Trainium2 NeuronCores (sparse token dispatch).

Strategy (hardcoded for T=2048, H=1024, E=16, I=512, IS=1024, top-k=2):
  - Expert-parallel: core c owns experts {2c, 2c+1}.  Router rows are
    permuted per core so the core's own experts are logit columns 0..1
    (keeps the SPMD program identical across cores).
  - Routing (logits + top-2) runs in fp32/fp32r so top-2 selection matches
    the fp32 reference.
  - All weights are pre-transposed and cast to bf16 on the HOST
    (wgT/wuT [H, I], wdT [I, H], swgT/swuT [H, ISS], swdT [ISS, H]) so the
    device does zero weight transposes and half the weight HBM traffic.
    A bf16 copy of x is also shipped for the token gather.
  - Sparse dispatch: per-expert token lists are built ON DEVICE via a PE
    triangular-matmul prefix-sum over the top-2 masks, then per-element
    one-hot matmuls produce the slot lists, combine weights and token ids.
  - Each expert gathers its <=C tokens (bf16 rows), PE-transposes to
    [H, C], computes SwiGLU (bf16 matmuls, fp32 PSUM), scales rows by the
    renormalized top-2 weight, and scatter-ACCUMULATES (SWDGE cce add)
    into a [T, H] bf16 partial that the shared-expert MLP (tensor-parallel
    over IS/8) initialized densely.
  - ReduceScatter(add) over 8 cores -> per-core [T/8, H] shard -> host
    concatenates.
"""

import sys

import numpy as np

if "/opt/trn_rl_repo" not in sys.path:
    sys.path.insert(0, "/opt/trn_rl_repo")

# ---- problem constants (hardcoded; kernel.py must be self-contained) ----
T, H, E, ID, IS = 2048, 1024, 16, 512, 1024
NCORES = 8
EPC = E // NCORES      # experts per core = 2
ISS = IS // NCORES     # shared intermediate slice = 128
TSH = T // NCORES      # output token shard = 256
P = 128
HC = H // P            # 8 h-chunks
TT = T // P            # 16 token tiles
NTS = T // 512         # 4 moving-free token slices
IC = ID // P           # 4 i-chunks per routed expert
HH = H // 512          # 2 moving-free h slices
C = 384                # per-expert token capacity (mean load is 256)
CT = C // P            # token tiles per expert list = 4
BIG = 1 << 20          # offset pushed past bounds_check -> scatter skips

_CACHE = {}


def _build_nc(n_iters: int = 1):
    from contextlib import ExitStack

    import concourse.bass as bass
    import concourse.mybir as mybir
    import concourse.tile as tile
    from concourse import bacc
    from concourse.masks import make_identity

    dt = mybir.dt
    f32, f32r, bf16 = dt.float32, dt.float32r, dt.bfloat16
    i32 = dt.int32
    AF = mybir.ActivationFunctionType
    OP = mybir.AluOpType

    nc = bacc.Bacc("TRN2", target_bir_lowering=False, debug=False,
                   num_devices=NCORES)

    # ---------------- kernel I/O ----------------
    x_d = nc.declare_dram_parameter("x", [T, H], f32, isOutput=False)
    xb_d = nc.declare_dram_parameter("xb", [T, H], bf16, isOutput=False)
    rw_d = nc.declare_dram_parameter("rw", [E, H], f32, isOutput=False)
    wgT_d = nc.declare_dram_parameter("wgT", [EPC, H, ID], bf16, isOutput=False)
    wuT_d = nc.declare_dram_parameter("wuT", [EPC, H, ID], bf16, isOutput=False)
    wdT_d = nc.declare_dram_parameter("wdT", [EPC, ID, H], bf16, isOutput=False)
    swgT_d = nc.declare_dram_parameter("swgT", [H, ISS], bf16, isOutput=False)
    swuT_d = nc.declare_dram_parameter("swuT", [H, ISS], bf16, isOutput=False)
    swdT_d = nc.declare_dram_parameter("swdT", [ISS, H], bf16, isOutput=False)
    out_d = nc.declare_dram_parameter("out", [TSH, H], f32, isOutput=True)

    with tile.TileContext(nc) as tc, ExitStack() as ctx:
        sb = ctx.enter_context(tc.tile_pool(name="sb", bufs=1))
        wst_p = ctx.enter_context(tc.tile_pool(name="wst", bufs=2))
        small_p = ctx.enter_context(tc.tile_pool(name="small", bufs=2))
        dram_p = ctx.enter_context(tc.tile_pool(name="dram", bufs=1, space="DRAM"))
        pp_mm = ctx.enter_context(tc.tile_pool(name="pp_mm", bufs=2, space="PSUM"))
        pp_tb = ctx.enter_context(tc.tile_pool(name="pp_tb", bufs=2, space="PSUM"))
        pp_tf = ctx.enter_context(tc.tile_pool(name="pp_tf", bufs=2, space="PSUM"))
        pp_log = ctx.enter_context(tc.tile_pool(name="pp_log", bufs=2, space="PSUM"))

        # DRAM scratch (double-buffered so iteration i+1's writes overlap
        # iteration i's ReduceScatter)
        partials = [dram_p.tile([T, H], bf16, name=f"partial{i}") for i in range(2)]
        rs_outs = [dram_p.tile([TSH, H], bf16, name=f"rs_out{i}") for i in range(2)]

        # ---------------- constants ----------------
        ident_b = sb.tile([P, P], bf16, name="ident_b")
        make_identity(nc, ident_b[:])
        ident_f = sb.tile([P, P], f32, name="ident_f")
        make_identity(nc, ident_f[:])
        # TRI[q, p] = 1 if q < p  (strict prefix over partitions)
        tri = sb.tile([P, P], f32, name="tri")
        nc.gpsimd.memset(tri[:], 0.0)
        nc.gpsimd.affine_select(
            out=tri[:], in_=tri[:], compare_op=OP.is_ge, fill=1.0,
            base=0, pattern=[[-1, P]], channel_multiplier=1)
        ones_row = sb.tile([1, P], f32, name="ones_row")
        nc.gpsimd.memset(ones_row[:], 1.0)
        ones_col = sb.tile([P, 1], f32, name="ones_col")
        nc.gpsimd.memset(ones_col[:], 1.0)
        # slot indices 0..C-1 (int32) and token-id columns (fp32)
        slot_i = sb.tile([P, C], i32, name="slot_i")
        nc.gpsimd.iota(slot_i[:], pattern=[[1, C]], base=0,
                       channel_multiplier=0)
        ids_p_i = sb.tile([P, 1], i32, name="ids_p_i")
        nc.gpsimd.iota(ids_p_i[:], pattern=[[0, 1]], base=0,
                       channel_multiplier=1)
        ids_p = sb.tile([P, 1], bf16, name="ids_p")
        nc.vector.tensor_copy(ids_p[:], ids_p_i[:])
        ids_t_i = sb.tile([P, TT], i32, name="ids_t_i")
        nc.gpsimd.iota(ids_t_i[:], pattern=[[1, TT]], base=0,
                       channel_multiplier=0)
        ids_t = sb.tile([P, TT], bf16, name="ids_t")
        nc.vector.tensor_copy(ids_t[:], ids_t_i[:])

        for _it in range(n_iters):
            partial = partials[_it % 2]
            rs_out = rs_outs[_it % 2]
            # shared-expert weights: no deps, issue DMAs first
            swgT = wst_p.tile([P, HC, ISS], bf16, tag="swst")
            nc.gpsimd.dma_start(out=swgT[:],
                                in_=swgT_d.rearrange("(a p) i -> p a i", p=P))
            swuT = wst_p.tile([P, HC, ISS], bf16, tag="swst")
            nc.gpsimd.dma_start(out=swuT[:],
                                in_=swuT_d.rearrange("(a p) i -> p a i", p=P))
            swdT = wst_p.tile([ISS, H], bf16, tag="swst")
            nc.gpsimd.dma_start(out=swdT[:], in_=swdT_d[:])

            # bf16 transposed x (for shared expert) + routing products
            xT = sb.tile([P, HC, T], bf16, name="xT")
            log_tm = sb.tile([P, TT, E], f32, name="log_tm")
            cw = sb.tile([P, TT, E], f32, name="cw")
            mk = sb.tile([P, TT, E], f32, name="mk")

            # ---- phase 1: x load, transpose, slab-wise routing ----
            # router weights -> [H, E] fp32r
            rw_st = sb.tile([E, H], f32, name="rw_st")
            nc.sync.dma_start(out=rw_st[:], in_=rw_d[:])
            rwT = sb.tile([P, HC, E], f32r, name="rwT")
            for hc in range(HC):
                pt = pp_tf.tile([P, 512], f32, tag="ptf")
                nc.tensor.transpose(pt[:, :E], rw_st[:, hc * P:(hc + 1) * P],
                                    ident_f[:E, :E])
                nc.vector.tensor_copy(rwT[:, hc, :], pt[:, :E])

            logT = sb.tile([E, T], f32, name="logT")
            with tc.tile_pool(name="xstage", bufs=2) as xs_p:
                for s in range(4):  # slabs of 512 tokens
                    ssl = slice(s * 512, (s + 1) * 512)
                    xslab = xs_p.tile([P, 4, H], f32, tag="xslab", bufs=2)
                    nc.sync.dma_start(
                        out=xslab[:],
                        in_=x_d[ssl, :].rearrange("(a p) h -> p a h", p=P))
                    xT32s = xs_p.tile([P, HC, 512], f32r, tag="xT32s", bufs=1)
                    for hc in range(HC):
                        pt = pp_tf.tile([P, 512], f32, tag="ptf")
                        for k in range(4):
                            nc.tensor.transpose(
                                pt[:, k * P:(k + 1) * P],
                                xslab[:, k, hc * P:(hc + 1) * P], ident_f[:])
                        nc.vector.tensor_copy(xT32s[:, hc, :], pt[:])
                        nc.scalar.copy(xT[:, hc, ssl], pt[:])
                    pl = pp_log.tile([E, 512], f32, tag="plog")
                    for hc in range(HC):
                        nc.tensor.matmul(
                            pl[:], rwT[:, hc, :], xT32s[:, hc, :],
                            start=(hc == 0), stop=(hc == HC - 1))
                    nc.scalar.copy(logT[:, ssl], pl[:])

            for tt in range(TT):
                pt = pp_tf.tile([P, 512], f32, tag="ptf")
                nc.tensor.transpose(pt[:, :E], logT[:, tt * P:(tt + 1) * P],
                                    ident_f[:E, :E])
                nc.vector.tensor_copy(log_tm[:, tt, :], pt[:, :E])

            # ---- top-2 -> combine weights cw + mask mk ----
            maxs = sb.tile([P, TT, 8], f32, name="maxs")
            for tt in range(TT):
                nc.vector.max(maxs[:, tt, :], log_tm[:, tt, :])
            d2 = sb.tile([P, TT], f32, name="d2")
            nc.vector.tensor_sub(d2[:], maxs[:, :, 1], maxs[:, :, 0])
            w2 = sb.tile([P, TT], f32, name="w2")
            nc.scalar.activation(w2[:], d2[:], AF.Exp)
            nc.vector.tensor_scalar_add(w2[:], w2[:], 1.0)
            rr = sb.tile([P, TT], f32, name="rr")
            nc.vector.reciprocal(rr[:], w2[:])

            dd = sb.tile([P, TT, E], f32, name="dd")
            nc.vector.tensor_sub(dd[:], log_tm[:],
                                 maxs[:, :, 0:1].to_broadcast([P, TT, E]))
            expd = sb.tile([P, TT, E], f32, name="expd")
            nc.scalar.activation(expd[:], dd[:], AF.Exp)
            nc.vector.tensor_tensor(
                out=mk[:], in0=log_tm[:],
                in1=maxs[:, :, 1:2].to_broadcast([P, TT, E]), op=OP.is_ge)
            nc.vector.tensor_mul(cw[:], expd[:], mk[:])
            nc.vector.tensor_mul(cw[:], cw[:],
                                 rr[:, :, None].to_broadcast([P, TT, E]))

            # ---- dispatch: positions via PE prefix-sum over local masks ----
            # per-tile totals, (tt, e) interleaved, on partition 0
            ptot = pp_log.tile([1, TT * EPC], f32, tag="plog")
            for tt in range(TT):
                nc.tensor.matmul(ptot[:, tt * EPC:(tt + 1) * EPC], ones_col[:],
                                 mk[:, tt, 0:EPC], start=True, stop=True)
            tot_row = sb.tile([1, TT, EPC], f32, name="tot_row")
            nc.vector.tensor_copy(tot_row[:], ptot[:])
            totE = sb.tile([1, EPC, TT], f32, name="totE")
            nc.vector.tensor_copy(totE[:], tot_row[:].rearrange("o t e -> o e t"))
            inclE = sb.tile([1, EPC, TT], f32, name="inclE")
            for e in range(EPC):
                nc.vector.tensor_tensor_scan(inclE[:, e, :], totE[:, e, :],
                                             totE[:, e, :], 0.0,
                                             op0=OP.add, op1=OP.bypass)
            exclE = sb.tile([1, EPC, TT], f32, name="exclE")
            nc.vector.tensor_sub(exclE[:], inclE[:], totE[:])

            pos = sb.tile([P, TT, EPC], f32, name="pos")
            for tq in range(4):
                pp = pp_tf.tile([P, 512], f32, tag="ptf")
                for k in range(4):
                    tt = tq * 4 + k
                    sl = slice(k * EPC, (k + 1) * EPC)
                    nc.tensor.matmul(pp[:, sl], tri[:], mk[:, tt, 0:EPC],
                                     start=True, stop=False)
                    nc.tensor.matmul(
                        pp[:, sl], ones_row[:],
                        exclE[:, :, tt:tt + 1].rearrange("o e t -> o (t e)"),
                        start=False, stop=True)
                nc.vector.tensor_copy(
                    pos[:, tq * 4:(tq + 1) * 4, :], pp[:, :4 * EPC])

            # ---- shared expert gate/up (PE) overlaps dispatch chain (DVE) ----
            acts_s = small_p.tile([P, T], bf16, tag="acts_s", bufs=1)
            for ts in range(NTS):
                tsl = slice(ts * 512, (ts + 1) * 512)
                pg = pp_mm.tile([P, 512], f32, tag="mm")
                pu = pp_mm.tile([P, 512], f32, tag="mm")
                for hc in range(HC):
                    nc.tensor.matmul(pg[:], swgT[:, hc, :], xT[:, hc, tsl],
                                     start=(hc == 0), stop=(hc == HC - 1))
                for hc in range(HC):
                    nc.tensor.matmul(pu[:], swuT[:, hc, :], xT[:, hc, tsl],
                                     start=(hc == 0), stop=(hc == HC - 1))
                sg = small_p.tile([P, 512], bf16, tag="sg")
                nc.scalar.activation(sg[:], pg[:], AF.Silu)
                nc.vector.tensor_tensor(out=acts_s[:, tsl], in0=sg[:],
                                        in1=pu[:], op=OP.mult)

            # ---- build per-expert slot lists via one-hot permutation matmuls ----
            pos_i = sb.tile([P, TT, EPC], i32, name="pos_i")
            nc.vector.tensor_copy(pos_i[:], pos[:])
            mk_i = sb.tile([P, TT, EPC], i32, name="mk_i")
            nc.vector.tensor_copy(mk_i[:], mk[:, :, 0:EPC])
            # posm = pos + (1-mask)*BIG  (masked-out tokens match no slot)
            drop = sb.tile([P, TT, EPC], i32, name="drop")
            nc.vector.tensor_scalar(drop[:], mk_i[:], -BIG, BIG,
                                    op0=OP.mult, op1=OP.add)
            posm = sb.tile([P, TT, EPC], i32, name="posm")
            nc.vector.tensor_add(posm[:], pos_i[:], drop[:])

            # rhs records [id%128, id//128, weight] per (expert, tile), bf16
            rec = sb.tile([P, EPC, TT, 3], bf16, name="rec")
            for e in range(EPC):
                nc.vector.tensor_copy(rec[:, e, :, 0],
                                      ids_p[:].to_broadcast([P, TT]))
                nc.vector.tensor_copy(rec[:, e, :, 1], ids_t[:])
                nc.vector.tensor_copy(rec[:, e, :, 2], cw[:, :, e])

            # lists_T[:, e, :] = rec_e^T @ onehot  ->  [3, C] per expert
            lists_T = sb.tile([3, EPC, C], f32, name="lists_T")
            for e in range(EPC):
                pl2 = pp_log.tile([3, C], f32, tag="plog")
                for tt in range(TT):
                    oh = small_p.tile([P, C], bf16, tag="oh")
                    nc.vector.tensor_tensor(
                        out=oh[:], in0=posm[:, tt, e:e + 1].to_broadcast([P, C]),
                        in1=slot_i[:], op=OP.is_equal)
                    nc.tensor.matmul(pl2[:], rec[:, e, tt, :], oh[:],
                                     start=(tt == 0), stop=(tt == TT - 1))
                nc.vector.tensor_copy(lists_T[:, e, :], pl2[:])

            # slot-major columns: [128, EPC, CT, 3]
            lists = sb.tile([P, EPC, CT, 3], f32, name="lists")
            for e in range(EPC):
                for ct in range(CT):
                    pt = pp_tf.tile([P, 512], f32, tag="ptf")
                    nc.tensor.transpose(
                        pt[:, :3], lists_T[:, e, ct * P:(ct + 1) * P],
                        ident_f[:3, :3])
                    nc.vector.tensor_copy(lists[:, e, ct, :], pt[:, :3])

            idx32_sb = sb.tile([P, EPC, CT], i32, name="idx32_sb")
            hi_i = sb.tile([P, EPC, CT], i32, name="hi_i")
            nc.vector.tensor_copy(hi_i[:], lists[:, :, :, 1])
            nc.vector.tensor_scalar(hi_i[:], hi_i[:], P, None, op0=OP.mult)
            nc.vector.tensor_copy(idx32_sb[:], lists[:, :, :, 0])
            nc.vector.tensor_add(idx32_sb[:], idx32_sb[:], hi_i[:])
            w_sb = sb.tile([P, EPC, CT], f32, name="w_sb")
            nc.vector.tensor_copy(w_sb[:], lists[:, :, :, 2])

            # ---- shared expert down-proj -> dense partial init ----
            for tt in range(TT):
                ys = small_p.tile([P, H], bf16, tag="ys")
                for hh in range(HH):
                    hsl = slice(hh * 512, (hh + 1) * 512)
                    py = pp_mm.tile([P, 512], f32, tag="mm")
                    nc.tensor.matmul(py[:], acts_s[:, tt * P:(tt + 1) * P],
                                     swdT[:, hsl], start=True, stop=True)
                    nc.scalar.copy(ys[:, hsl], py[:])
                nc.sync.dma_start(out=partial[tt * P:(tt + 1) * P, :], in_=ys[:])

            # ---- routed experts (sparse, capacity C) ----
            for e in range(EPC):
                wgT = wst_p.tile([P, HC, ID], bf16, tag="wst")
                nc.gpsimd.dma_start(
                    out=wgT[:], in_=wgT_d[e].rearrange("(a p) i -> p a i", p=P))
                wuT = wst_p.tile([P, HC, ID], bf16, tag="wst")
                nc.gpsimd.dma_start(
                    out=wuT[:], in_=wuT_d[e].rearrange("(a p) i -> p a i", p=P))
                wdT = wst_p.tile([P, IC, H], bf16, tag="wst")
                nc.gpsimd.dma_start(
                    out=wdT[:], in_=wdT_d[e].rearrange("(a p) h -> p a h", p=P))

                # gather this expert's tokens (bf16 rows), then PE-transpose
                xg = small_p.tile([P, CT, H], bf16, tag="xg", bufs=1)
                for ct in range(CT):
                    nc.gpsimd.indirect_dma_start(
                        out=xg[:, ct, :], out_offset=None,
                        in_=xb_d[:], in_offset=bass.IndirectOffsetOnAxis(
                            ap=idx32_sb[:, e, ct:ct + 1], axis=0))
                xgT = small_p.tile([P, HC, C], bf16, tag="xgT")
                for ct in range(CT):
                    for hq in range(2):
                        pt = pp_tb.tile([P, 512], bf16, tag="ptb")
                        for k in range(4):
                            hc = hq * 4 + k
                            nc.tensor.transpose(
                                pt[:, k * P:(k + 1) * P],
                                xg[:, ct, hc * P:(hc + 1) * P], ident_b[:])
                        nc.vector.tensor_copy(
                            xgT[:, hq * 4:(hq + 1) * 4, ct * P:(ct + 1) * P],
                            pt[:].rearrange("p (a b) -> p a b", a=4))

                # gate/up + silu: act_fm [i, C]
                act_fm = small_p.tile([P, IC, C], bf16, tag="act_fm", bufs=1)
                for ic in range(IC):
                    isl = slice(ic * P, (ic + 1) * P)
                    pg = pp_mm.tile([P, C], f32, tag="mm")
                    pu = pp_mm.tile([P, C], f32, tag="mm")
                    for hc in range(HC):
                        nc.tensor.matmul(pg[:], wgT[:, hc, isl], xgT[:, hc, :],
                                         start=(hc == 0), stop=(hc == HC - 1))
                    for hc in range(HC):
                        nc.tensor.matmul(pu[:], wuT[:, hc, isl], xgT[:, hc, :],
                                         start=(hc == 0), stop=(hc == HC - 1))
                    sg = small_p.tile([P, C], bf16, tag="sg")
                    nc.scalar.activation(sg[:], pg[:], AF.Silu)
                    nc.vector.tensor_tensor(out=act_fm[:, ic, :], in0=sg[:],
                                            in1=pu[:], op=OP.mult)

                # down-proj + weight + scatter-accumulate into partial
                for ct in range(CT):
                    yw = small_p.tile([P, H], bf16, tag="yw")
                    for hh in range(HH):
                        hsl = slice(hh * 512, (hh + 1) * 512)
                        py = pp_mm.tile([P, 512], f32, tag="mm")
                        for ic in range(IC):
                            nc.tensor.matmul(
                                py[:], act_fm[:, ic, ct * P:(ct + 1) * P],
                                wdT[:, ic, hsl],
                                start=(ic == 0), stop=(ic == IC - 1))
                        nc.scalar.mul(yw[:, hsl], py[:], w_sb[:, e, ct:ct + 1])
                    nc.gpsimd.indirect_dma_start(
                        out=partial[:], out_offset=bass.IndirectOffsetOnAxis(
                            ap=idx32_sb[:, e, ct:ct + 1], axis=0),
                        in_=yw[:], in_offset=None,
                        compute_op=OP.add)

            # ---- combine: ReduceScatter(add) over the 8 cores ----
            nc.gpsimd.collective_compute(
                "ReduceScatter", OP.add,
                replica_groups=[list(range(NCORES))],
                ins=[partial[:]], outs=[rs_out[:]])
            nc.gpsimd.dma_start(out=out_d[:], in_=rs_out[:])

    nc.compile()
    return nc


def _get_nc(n_iters: int = 1):
    key = ("nc", n_iters)
    if key not in _CACHE:
        _CACHE[key] = _build_nc(n_iters)
    return _CACHE[key]


def make_in_maps(x, router_w, wg, wu, wd, sw_gate, sw_up, sw_down):
    """Build the per-core input maps (host-side sharding + layout prep)."""
    import ml_dtypes

    bf16 = ml_dtypes.bfloat16
    x = np.ascontiguousarray(x, dtype=np.float32)
    xb = np.ascontiguousarray(x.astype(bf16))
    wgT = np.ascontiguousarray(np.transpose(wg, (0, 2, 1)).astype(bf16))
    wuT = np.ascontiguousarray(np.transpose(wu, (0, 2, 1)).astype(bf16))
    wdT = np.ascontiguousarray(np.transpose(wd, (0, 2, 1)).astype(bf16))
    swgT = np.ascontiguousarray(sw_gate.T.astype(bf16))
    swuT = np.ascontiguousarray(sw_up.T.astype(bf16))
    swdT = np.ascontiguousarray(sw_down.T.astype(bf16))
    in_maps = []
    for c in range(NCORES):
        own = [EPC * c + k for k in range(EPC)]
        others = [e for e in range(E) if e not in own]
        perm = own + others
        in_maps.append({
            "x": x,
            "xb": xb,
            "rw": np.ascontiguousarray(router_w[perm], dtype=np.float32),
            "wgT": wgT[own],
            "wuT": wuT[own],
            "wdT": wdT[own],
            "swgT": np.ascontiguousarray(swgT[:, c * ISS:(c + 1) * ISS]),
            "swuT": np.ascontiguousarray(swuT[:, c * ISS:(c + 1) * ISS]),
            "swdT": np.ascontiguousarray(swdT[c * ISS:(c + 1) * ISS]),
        })
    return in_maps


def kernel(x, router_w, wg, wu, wd, sw_gate, sw_up, sw_down):
    from concourse.bass_utils import run_bass_kernel_spmd

    nc = _get_nc()
    in_maps = make_in_maps(x, router_w, wg, wu, wd, sw_gate, sw_up, sw_down)
    res = run_bass_kernel_spmd(nc, in_maps, list(range(NCORES))).results
    out = np.concatenate([res[c]["out"] for c in range(NCORES)], axis=0)
    return out.astype(np.float32)


if __name__ == "__main__":
    nc = _build_nc()
    print("built ok")


# revision 9
# speedup vs baseline: 1.3527x; 1.3527x over previous
"""DeepseekMoE on 8 Trainium2 NeuronCores (sparse token dispatch).

Strategy (hardcoded for T=2048, H=1024, E=16, I=512, IS=1024, top-k=2):
  - Expert-parallel: core c owns experts {2c, 2c+1}.  Router rows are
    permuted per core so the core's own experts are logit columns 0..1
    (keeps the SPMD program identical across cores).
  - Routing (logits + top-2) runs in fp32/fp32r so top-2 selection matches
    the fp32 reference.
  - All weights are pre-transposed and cast to bf16 on the HOST
    (wgT/wuT [H, I], wdT [I, H], swgT/swuT [H, ISS], swdT [ISS, H]) so the
    device does zero weight transposes and half the weight HBM traffic.
    A bf16 copy of x is also shipped for the token gather.
  - Sparse dispatch: per-expert token lists are built ON DEVICE via a PE
    triangular-matmul prefix-sum over the top-2 masks, then per-element
    one-hot matmuls produce the slot lists, combine weights and token ids.
  - Each expert gathers its <=C tokens (bf16 rows), PE-transposes to
    [H, C], computes SwiGLU (bf16 matmuls, fp32 PSUM), scales rows by the
    renormalized top-2 weight, and scatter-ACCUMULATES (SWDGE cce add)
    into a [T, H] bf16 partial that the shared-expert MLP (tensor-parallel
    over IS/8) initialized densely.
  - ReduceScatter(add) over 8 cores -> per-core [T/8, H] shard -> host
    concatenates.
"""

import sys

import numpy as np

if "/opt/trn_rl_repo" not in sys.path:
    sys.path.insert(0, "/opt/trn_rl_repo")

# ---- problem constants (hardcoded; kernel.py must be self-contained) ----
T, H, E, ID, IS = 2048, 1024, 16, 512, 1024
NCORES = 8
EPC = E // NCORES      # experts per core = 2
ISS = IS // NCORES     # shared intermediate slice = 128
TSH = T // NCORES      # output token shard = 256
P = 128
HC = H // P            # 8 h-chunks
TT = T // P            # 16 token tiles
NTS = T // 512         # 4 moving-free token slices
IC = ID // P           # 4 i-chunks per routed expert
HH = H // 512          # 2 moving-free h slices
C = 384                # per-expert token capacity (mean load is 256)
CT = C // P            # token tiles per expert list = 4
BIG = 1 << 20          # offset pushed past bounds_check -> scatter skips

_CACHE = {}


def _build_nc(n_iters: int = 1):
    from contextlib import ExitStack

    import concourse.bass as bass
    import concourse.mybir as mybir
    import concourse.tile as tile
    from concourse import bacc
    from concourse.masks import make_identity

    dt = mybir.dt
    f32, f32r, bf16 = dt.float32, dt.float32r, dt.bfloat16
    i32 = dt.int32
    AF = mybir.ActivationFunctionType
    OP = mybir.AluOpType

    nc = bacc.Bacc("TRN2", target_bir_lowering=False, debug=False,
                   num_devices=NCORES)

    # ---------------- kernel I/O ----------------
    x_d = nc.declare_dram_parameter("x", [T, H], f32, isOutput=False)
    xb_d = nc.declare_dram_parameter("xb", [T, H], bf16, isOutput=False)
    rw_d = nc.declare_dram_parameter("rw", [E, H], f32, isOutput=False)
    wgT_d = nc.declare_dram_parameter("wgT", [EPC, H, ID], bf16, isOutput=False)
    wuT_d = nc.declare_dram_parameter("wuT", [EPC, H, ID], bf16, isOutput=False)
    wdT_d = nc.declare_dram_parameter("wdT", [EPC, ID, H], bf16, isOutput=False)
    swgT_d = nc.declare_dram_parameter("swgT", [H, ISS], bf16, isOutput=False)
    swuT_d = nc.declare_dram_parameter("swuT", [H, ISS], bf16, isOutput=False)
    swdT_d = nc.declare_dram_parameter("swdT", [ISS, H], bf16, isOutput=False)
    out_d = nc.declare_dram_parameter("out", [TSH, H], f32, isOutput=True)

    with tile.TileContext(nc) as tc, ExitStack() as ctx:
        sb = ctx.enter_context(tc.tile_pool(name="sb", bufs=1))
        wst_p = ctx.enter_context(tc.tile_pool(name="wst", bufs=2))
        small_p = ctx.enter_context(tc.tile_pool(name="small", bufs=2))
        dram_p = ctx.enter_context(tc.tile_pool(name="dram", bufs=1, space="DRAM"))
        pp_mm = ctx.enter_context(tc.tile_pool(name="pp_mm", bufs=2, space="PSUM"))
        pp_tb = ctx.enter_context(tc.tile_pool(name="pp_tb", bufs=2, space="PSUM"))
        pp_tf = ctx.enter_context(tc.tile_pool(name="pp_tf", bufs=2, space="PSUM"))
        pp_log = ctx.enter_context(tc.tile_pool(name="pp_log", bufs=2, space="PSUM"))

        # DRAM scratch (double-buffered so iteration i+1's writes overlap
        # iteration i's ReduceScatter)
        partials = [dram_p.tile([T, H], bf16, name=f"partial{i}") for i in range(2)]
        rs_outs = [dram_p.tile([TSH, H], bf16, name=f"rs_out{i}") for i in range(2)]

        # ---------------- constants ----------------
        ident_b = sb.tile([P, P], bf16, name="ident_b")
        make_identity(nc, ident_b[:])
        ident_f = sb.tile([P, P], f32, name="ident_f")
        make_identity(nc, ident_f[:])
        # TRI[q, p] = 1 if q < p  (strict prefix over partitions)
        tri = sb.tile([P, P], f32, name="tri")
        nc.gpsimd.memset(tri[:], 0.0)
        nc.gpsimd.affine_select(
            out=tri[:], in_=tri[:], compare_op=OP.is_ge, fill=1.0,
            base=0, pattern=[[-1, P]], channel_multiplier=1)
        ones_row = sb.tile([1, P], f32, name="ones_row")
        nc.gpsimd.memset(ones_row[:], 1.0)
        ones_col = sb.tile([P, 1], f32, name="ones_col")
        nc.gpsimd.memset(ones_col[:], 1.0)
        # slot indices 0..C-1 (int32) and token-id columns (fp32)
        slot_i = sb.tile([P, C], i32, name="slot_i")
        nc.gpsimd.iota(slot_i[:], pattern=[[1, C]], base=0,
                       channel_multiplier=0)
        ids_p_i = sb.tile([P, 1], i32, name="ids_p_i")
        nc.gpsimd.iota(ids_p_i[:], pattern=[[0, 1]], base=0,
                       channel_multiplier=1)
        ids_p = sb.tile([P, 1], bf16, name="ids_p")
        nc.vector.tensor_copy(ids_p[:], ids_p_i[:])
        ids_t_i = sb.tile([P, TT], i32, name="ids_t_i")
        nc.gpsimd.iota(ids_t_i[:], pattern=[[1, TT]], base=0,
                       channel_multiplier=0)
        ids_t = sb.tile([P, TT], bf16, name="ids_t")
        nc.vector.tensor_copy(ids_t[:], ids_t_i[:])

        for _it in range(n_iters):
            partial = partials[_it % 2]
            rs_out = rs_outs[_it % 2]
            # shared-expert weights: no deps, issue DMAs first
            swgT = wst_p.tile([P, HC, ISS], bf16, tag="swst")
            nc.gpsimd.dma_start(out=swgT[:],
                                in_=swgT_d.rearrange("(a p) i -> p a i", p=P))
            swuT = wst_p.tile([P, HC, ISS], bf16, tag="swst")
            nc.gpsimd.dma_start(out=swuT[:],
                                in_=swuT_d.rearrange("(a p) i -> p a i", p=P))
            swdT = wst_p.tile([ISS, H], bf16, tag="swst")
            nc.gpsimd.dma_start(out=swdT[:], in_=swdT_d[:])

            # bf16 transposed x (for shared expert) + routing products
            xT = sb.tile([P, HC, T], bf16, name="xT")
            log_tm = sb.tile([P, TT, E], f32, name="log_tm")
            cw = sb.tile([P, TT, E], f32, name="cw")
            mk = sb.tile([P, TT, E], f32, name="mk")

            # ---- phase 1: x load, transpose, slab-wise routing ----
            # router weights -> [H, E] fp32r
            rw_st = sb.tile([E, H], f32, name="rw_st")
            nc.sync.dma_start(out=rw_st[:], in_=rw_d[:])
            rwT = sb.tile([P, HC, E], f32r, name="rwT")
            for hc in range(HC):
                pt = pp_tf.tile([P, 512], f32, tag="ptf")
                nc.tensor.transpose(pt[:, :E], rw_st[:, hc * P:(hc + 1) * P],
                                    ident_f[:E, :E])
                nc.vector.tensor_copy(rwT[:, hc, :], pt[:, :E])

            logT = sb.tile([E, T], f32, name="logT")
            with tc.tile_pool(name="xstage", bufs=2) as xs_p:
                for s in range(4):  # slabs of 512 tokens
                    ssl = slice(s * 512, (s + 1) * 512)
                    xslab = xs_p.tile([P, 4, H], f32, tag="xslab", bufs=2)
                    nc.sync.dma_start(
                        out=xslab[:],
                        in_=x_d[ssl, :].rearrange("(a p) h -> p a h", p=P))
                    xT32s = xs_p.tile([P, HC, 512], f32r, tag="xT32s", bufs=1)
                    for hc in range(HC):
                        pt = pp_tf.tile([P, 512], f32, tag="ptf")
                        for k in range(4):
                            nc.tensor.transpose(
                                pt[:, k * P:(k + 1) * P],
                                xslab[:, k, hc * P:(hc + 1) * P], ident_f[:])
                        nc.vector.tensor_copy(xT32s[:, hc, :], pt[:])
                        nc.scalar.copy(xT[:, hc, ssl], pt[:])
                    pl = pp_log.tile([E, 512], f32, tag="plog")
                    for hc in range(HC):
                        nc.tensor.matmul(
                            pl[:], rwT[:, hc, :], xT32s[:, hc, :],
                            start=(hc == 0), stop=(hc == HC - 1))
                    nc.scalar.copy(logT[:, ssl], pl[:])

            for tt in range(TT):
                pt = pp_tf.tile([P, 512], f32, tag="ptf")
                nc.tensor.transpose(pt[:, :E], logT[:, tt * P:(tt + 1) * P],
                                    ident_f[:E, :E])
                nc.vector.tensor_copy(log_tm[:, tt, :], pt[:, :E])

            # ---- top-2 -> combine weights cw + mask mk ----
            maxs = sb.tile([P, TT, 8], f32, name="maxs")
            for tt in range(TT):
                nc.vector.max(maxs[:, tt, :], log_tm[:, tt, :])
            d2 = sb.tile([P, TT], f32, name="d2")
            nc.vector.tensor_sub(d2[:], maxs[:, :, 1], maxs[:, :, 0])
            w2 = sb.tile([P, TT], f32, name="w2")
            nc.scalar.activation(w2[:], d2[:], AF.Exp)
            nc.vector.tensor_scalar_add(w2[:], w2[:], 1.0)
            rr = sb.tile([P, TT], f32, name="rr")
            nc.vector.reciprocal(rr[:], w2[:])

            dd = sb.tile([P, TT, E], f32, name="dd")
            nc.vector.tensor_sub(dd[:], log_tm[:],
                                 maxs[:, :, 0:1].to_broadcast([P, TT, E]))
            expd = sb.tile([P, TT, E], f32, name="expd")
            nc.scalar.activation(expd[:], dd[:], AF.Exp)
            nc.vector.tensor_tensor(
                out=mk[:], in0=log_tm[:],
                in1=maxs[:, :, 1:2].to_broadcast([P, TT, E]), op=OP.is_ge)
            nc.vector.tensor_mul(cw[:], expd[:], mk[:])
            nc.vector.tensor_mul(cw[:], cw[:],
                                 rr[:, :, None].to_broadcast([P, TT, E]))

            # ---- dispatch: positions via PE prefix-sum over local masks ----
            # per-tile totals, (tt, e) interleaved, on partition 0
            ptot = pp_log.tile([1, TT * EPC], f32, tag="plog")
            for tt in range(TT):
                nc.tensor.matmul(ptot[:, tt * EPC:(tt + 1) * EPC], ones_col[:],
                                 mk[:, tt, 0:EPC], start=True, stop=True)
            tot_row = sb.tile([1, TT, EPC], f32, name="tot_row")
            nc.vector.tensor_copy(tot_row[:], ptot[:])
            totE = sb.tile([1, EPC, TT], f32, name="totE")
            nc.vector.tensor_copy(totE[:], tot_row[:].rearrange("o t e -> o e t"))
            inclE = sb.tile([1, EPC, TT], f32, name="inclE")
            for e in range(EPC):
                nc.vector.tensor_tensor_scan(inclE[:, e, :], totE[:, e, :],
                                             totE[:, e, :], 0.0,
                                             op0=OP.add, op1=OP.bypass)
            exclE = sb.tile([1, EPC, TT], f32, name="exclE")
            nc.vector.tensor_sub(exclE[:], inclE[:], totE[:])

            pos = sb.tile([P, TT, EPC], f32, name="pos")
            for tq in range(4):
                pp = pp_tf.tile([P, 512], f32, tag="ptf")
                for k in range(4):
                    tt = tq * 4 + k
                    sl = slice(k * EPC, (k + 1) * EPC)
                    nc.tensor.matmul(pp[:, sl], tri[:], mk[:, tt, 0:EPC],
                                     start=True, stop=False)
                    nc.tensor.matmul(
                        pp[:, sl], ones_row[:],
                        exclE[:, :, tt:tt + 1].rearrange("o e t -> o (t e)"),
                        start=False, stop=True)
                nc.vector.tensor_copy(
                    pos[:, tq * 4:(tq + 1) * 4, :], pp[:, :4 * EPC])

            # ---- build per-expert slot lists via one-hot permutation matmuls ----
            pos_i = sb.tile([P, TT, EPC], i32, name="pos_i")
            nc.vector.tensor_copy(pos_i[:], pos[:])
            mk_i = sb.tile([P, TT, EPC], i32, name="mk_i")
            nc.vector.tensor_copy(mk_i[:], mk[:, :, 0:EPC])
            # posm = pos + (1-mask)*BIG  (masked-out tokens match no slot)
            drop = sb.tile([P, TT, EPC], i32, name="drop")
            nc.vector.tensor_scalar(drop[:], mk_i[:], -BIG, BIG,
                                    op0=OP.mult, op1=OP.add)
            posm = sb.tile([P, TT, EPC], i32, name="posm")
            nc.vector.tensor_add(posm[:], pos_i[:], drop[:])

            # rhs records [id%128, id//128, weight] per (expert, tile), bf16
            rec = sb.tile([P, EPC, TT, 3], bf16, name="rec")
            for e in range(EPC):
                nc.vector.tensor_copy(rec[:, e, :, 0],
                                      ids_p[:].to_broadcast([P, TT]))
                nc.vector.tensor_copy(rec[:, e, :, 1], ids_t[:])
                nc.vector.tensor_copy(rec[:, e, :, 2], cw[:, :, e])

            # lists_T[:, e, :] = rec_e^T @ onehot  ->  [3, C] per expert
            lists_T = sb.tile([3, EPC, C], f32, name="lists_T")
            for e in range(EPC):
                pl2 = pp_log.tile([3, C], f32, tag="plog")
                for tt in range(TT):
                    oh = small_p.tile([P, C], bf16, tag="oh")
                    nc.vector.tensor_tensor(
                        out=oh[:], in0=posm[:, tt, e:e + 1].to_broadcast([P, C]),
                        in1=slot_i[:], op=OP.is_equal)
                    nc.tensor.matmul(pl2[:], rec[:, e, tt, :], oh[:],
                                     start=(tt == 0), stop=(tt == TT - 1))
                nc.vector.tensor_copy(lists_T[:, e, :], pl2[:])

            # slot-major columns: [128, EPC, CT, 3]
            lists = sb.tile([P, EPC, CT, 3], f32, name="lists")
            for e in range(EPC):
                for ct in range(CT):
                    pt = pp_tf.tile([P, 512], f32, tag="ptf")
                    nc.tensor.transpose(
                        pt[:, :3], lists_T[:, e, ct * P:(ct + 1) * P],
                        ident_f[:3, :3])
                    nc.vector.tensor_copy(lists[:, e, ct, :], pt[:, :3])

            idx32_sb = sb.tile([P, EPC, CT], i32, name="idx32_sb")
            hi_i = sb.tile([P, EPC, CT], i32, name="hi_i")
            nc.vector.tensor_copy(hi_i[:], lists[:, :, :, 1])
            nc.vector.tensor_scalar(hi_i[:], hi_i[:], P, None, op0=OP.mult)
            nc.vector.tensor_copy(idx32_sb[:], lists[:, :, :, 0])
            nc.vector.tensor_add(idx32_sb[:], idx32_sb[:], hi_i[:])
            w_sb = sb.tile([P, EPC, CT], f32, name="w_sb")
            nc.vector.tensor_copy(w_sb[:], lists[:, :, :, 2])

            # ---- shared expert (TP slice of IS) -> dense partial init ----
            acts_s = small_p.tile([P, T], bf16, tag="acts_s", bufs=1)
            for ts in range(NTS):
                tsl = slice(ts * 512, (ts + 1) * 512)
                pg = pp_mm.tile([P, 512], f32, tag="mm")
                pu = pp_mm.tile([P, 512], f32, tag="mm")
                for hc in range(HC):
                    nc.tensor.matmul(pg[:], swgT[:, hc, :], xT[:, hc, tsl],
                                     start=(hc == 0), stop=(hc == HC - 1))
                for hc in range(HC):
                    nc.tensor.matmul(pu[:], swuT[:, hc, :], xT[:, hc, tsl],
                                     start=(hc == 0), stop=(hc == HC - 1))
                sg = small_p.tile([P, 512], bf16, tag="sg")
                nc.scalar.activation(sg[:], pg[:], AF.Silu)
                nc.vector.tensor_tensor(out=acts_s[:, tsl], in0=sg[:],
                                        in1=pu[:], op=OP.mult)

            for tt in range(TT):
                ys = small_p.tile([P, H], bf16, tag="ys")
                for hh in range(HH):
                    hsl = slice(hh * 512, (hh + 1) * 512)
                    py = pp_mm.tile([P, 512], f32, tag="mm")
                    nc.tensor.matmul(py[:], acts_s[:, tt * P:(tt + 1) * P],
                                     swdT[:, hsl], start=True, stop=True)
                    nc.scalar.copy(ys[:, hsl], py[:])
                nc.sync.dma_start(out=partial[tt * P:(tt + 1) * P, :], in_=ys[:])

            # ---- routed experts (sparse, capacity C) ----
            for e in range(EPC):
                wgT = wst_p.tile([P, HC, ID], bf16, tag="wst")
                nc.gpsimd.dma_start(
                    out=wgT[:], in_=wgT_d[e].rearrange("(a p) i -> p a i", p=P))
                wuT = wst_p.tile([P, HC, ID], bf16, tag="wst")
                nc.gpsimd.dma_start(
                    out=wuT[:], in_=wuT_d[e].rearrange("(a p) i -> p a i", p=P))
                wdT = wst_p.tile([P, IC, H], bf16, tag="wst")
                nc.gpsimd.dma_start(
                    out=wdT[:], in_=wdT_d[e].rearrange("(a p) h -> p a h", p=P))

                # gather this expert's tokens (bf16 rows), then PE-transpose
                xg = small_p.tile([P, CT, H], bf16, tag="xg", bufs=1)
                for ct in range(CT):
                    nc.gpsimd.indirect_dma_start(
                        out=xg[:, ct, :], out_offset=None,
                        in_=xb_d[:], in_offset=bass.IndirectOffsetOnAxis(
                            ap=idx32_sb[:, e, ct:ct + 1], axis=0))
                xgT = small_p.tile([P, HC, C], bf16, tag="xgT")
                for ct in range(CT):
                    for hq in range(2):
                        pt = pp_tb.tile([P, 512], bf16, tag="ptb")
                        for k in range(4):
                            hc = hq * 4 + k
                            nc.tensor.transpose(
                                pt[:, k * P:(k + 1) * P],
                                xg[:, ct, hc * P:(hc + 1) * P], ident_b[:])
                        nc.vector.tensor_copy(
                            xgT[:, hq * 4:(hq + 1) * 4, ct * P:(ct + 1) * P],
                            pt[:].rearrange("p (a b) -> p a b", a=4))

                # gate/up + silu: act_fm [i, C]
                act_fm = small_p.tile([P, IC, C], bf16, tag="act_fm", bufs=1)
                for ic in range(IC):
                    isl = slice(ic * P, (ic + 1) * P)
                    pg = pp_mm.tile([P, C], f32, tag="mm")
                    pu = pp_mm.tile([P, C], f32, tag="mm")
                    for hc in range(HC):
                        nc.tensor.matmul(pg[:], wgT[:, hc, isl], xgT[:, hc, :],
                                         start=(hc == 0), stop=(hc == HC - 1))
                    for hc in range(HC):
                        nc.tensor.matmul(pu[:], wuT[:, hc, isl], xgT[:, hc, :],
                                         start=(hc == 0), stop=(hc == HC - 1))
                    sg = small_p.tile([P, C], bf16, tag="sg")
                    nc.scalar.activation(sg[:], pg[:], AF.Silu)
                    nc.vector.tensor_tensor(out=act_fm[:, ic, :], in0=sg[:],
                                            in1=pu[:], op=OP.mult)

                # down-proj + weight + scatter-accumulate into partial
                for ct in range(CT):
                    yw = small_p.tile([P, H], bf16, tag="yw")
                    for hh in range(HH):
                        hsl = slice(hh * 512, (hh + 1) * 512)
                        py = pp_mm.tile([P, 512], f32, tag="mm")
                        for ic in range(IC):
                            nc.tensor.matmul(
                                py[:], act_fm[:, ic, ct * P:(ct + 1) * P],
                                wdT[:, ic, hsl],
                                start=(ic == 0), stop=(ic == IC - 1))
                        nc.scalar.mul(yw[:, hsl], py[:], w_sb[:, e, ct:ct + 1])
                    nc.gpsimd.indirect_dma_start(
                        out=partial[:], out_offset=bass.IndirectOffsetOnAxis(
                            ap=idx32_sb[:, e, ct:ct + 1], axis=0),
                        in_=yw[:], in_offset=None,
                        compute_op=OP.add)

            # ---- combine: ReduceScatter(add) over the 8 cores ----
            nc.gpsimd.collective_compute(
                "ReduceScatter", OP.add,
                replica_groups=[list(range(NCORES))],
                ins=[partial[:]], outs=[rs_out[:]])
            nc.gpsimd.dma_start(out=out_d[:], in_=rs_out[:])

    nc.compile()
    return nc


def _get_nc(n_iters: int = 1):
    key = ("nc", n_iters)
    if key not in _CACHE:
        _CACHE[key] = _build_nc(n_iters)
    return _CACHE[key]


def make_in_maps(x, router_w, wg, wu, wd, sw_gate, sw_up, sw_down):
    """Build the per-core input maps (host-side sharding + layout prep)."""
    import ml_dtypes

    bf16 = ml_dtypes.bfloat16
    x = np.ascontiguousarray(x, dtype=np.float32)
    xb = np.ascontiguousarray(x.astype(bf16))
    wgT = np.ascontiguousarray(np.transpose(wg, (0, 2, 1)).astype(bf16))
    wuT = np.ascontiguousarray(np.transpose(wu, (0, 2, 1)).astype(bf16))
    wdT = np.ascontiguousarray(np.transpose(wd, (0, 2, 1)).astype(bf16))
    swgT = np.ascontiguousarray(sw_gate.T.astype(bf16))
    swuT = np.ascontiguousarray(sw_up.T.astype(bf16))
    swdT = np.ascontiguousarray(sw_down.T.astype(bf16))
    in_maps = []
    for c in range(NCORES):
        own = [EPC * c + k for k in range(EPC)]
        others = [e for e in range(E) if e not in own]
        perm = own + others
        in_maps.append({
            "x": x,
            "xb": xb,
            "rw": np.ascontiguousarray(router_w[perm], dtype=np.float32),
            "wgT": wgT[own],
            "wuT": wuT[own],
            "wdT": wdT[own],
            "swgT": np.ascontiguousarray(swgT[:, c * ISS:(c + 1) * ISS]),
            "swuT": np.ascontiguousarray(swuT[:, c * ISS:(c + 1) * ISS]),
            "swdT": np.ascontiguousarray(swdT[c * ISS:(c + 1) * ISS]),
        })
    return in_maps


def kernel(x, router_w, wg, wu, wd, sw_gate, sw_up, sw_down):
    from concourse.bass_utils import run_bass_kernel_spmd

    nc = _get_nc()
    in_maps = make_in_maps(x, router_w, wg, wu, wd, sw_gate, sw_up, sw_down)
    res = run_bass_kernel_spmd(nc, in_maps, list(range(NCORES))).results
    out = np.concatenate([res[c]["out"] for c in range(NCORES)], axis=0)
    return out.astype(np.float32)


if __name__ == "__main__":
    nc = _build_nc()
    print("built ok")
